# revision 25
# baseline (speedup 1.0000x reference)
"""Trainium2 Bass kernel for an AttentionBlock (GroupNorm + MHSA + proj + residual).

Problem shapes (hardcoded): x [B=8, C=512, H=32, W=32], T = H*W = 1024,
NH=8 heads (head_dim 64), GroupNorm groups G=32, eps 1e-5.

Sharding: data-parallel over batch B across the 8 NeuronCores — one batch
element per core, no collectives.

Per-core dataflow (all layouts [partition, free]):
  x        [C, T]   4 sbuf tiles of [128, 1024] f32
  GroupNorm stats: per-tile row sums (DVE) / sums-of-squares (ACT Square with
           accum_out), group-summed across partitions with a tiny indicator
           matmul, rstd via Newton rsqrt on DVE, then per-channel scale/bias
           broadcast back with another tiny matmul.
  xn       [C, T]   = x*scale + bias (DVE tensor_scalar), bf16
  q,k = W_qk^T.T @ xn   -> bf16 tiles [128, 1024]
  vT  = xn.T @ WvT      -> vT tiles [128, 8*65] bf16 (col 64 of each head
                           block memset to 1.0: fused softmax-denominator)
  scoresT[s,t] = k_h^T q_h : K=64 matmuls, two heads (j) per pair tile.
  E = exp(scoresT/8)    -> bf16 sbuf (one ACT pass per [128, 1024] psum tile;
                           the last pair is split into t-halves so the tail
                           av/proj work overlaps the remaining exp stream)
  avT: out[t, j*65+d] = sum_s E_j[s,t] vT[s, h*65+d]  (psum [128, 130] per
                           (pair, t-tile); column 64 of each j-block is the
                           softmax denominator Z because of the vT ones col)
  normalize: rz = 1/Z (DVE reciprocal of the two Z columns), then per-j
                           tensor_scalar multiply psum -> aTn bf16 [t, c]
  transpose: XBAR dma_start_transpose per t-tile [128 t, 512 c] ->
                           a_all[p, pair, t] (the [c, t] layout proj needs)
  out = WpT.T @ a + x + bpe  (bpe folded into x mid-kernel on DVE; epilogue
                           is one DVE add psum+x) -> DMA out [C, T]
"""

import numpy as np
import ml_dtypes

import concourse.bacc as bacc
from concourse import mybir
from concourse.tile import TileContext
from concourse.bass_utils import run_bass_kernel_spmd

F32 = mybir.dt.float32
F32R = mybir.dt.float32r
BF16 = mybir.dt.bfloat16
AF = mybir.ActivationFunctionType
ALU = mybir.AluOpType
AX = mybir.AxisListType

B = 8
C = 512
H = W = 32
T = H * W            # 1024
NH = 8
HD = C // NH         # 64
G = 32               # groupnorm groups
GSZ = C // G         # 16 channels per group
EPS = 1e-5
NCT = C // 128       # 4 channel tiles
NTT = T // 128       # 8 token tiles
NP = NH // 2         # 4 head pairs
SCALE = 1.0 / np.sqrt(HD)   # 0.125
NELEM_GROUP = GSZ * T       # 16384 elements per group

# Schraudolph-style exp for the DVE-offloaded score tiles: the bf16 bit
# pattern of exp(x) is approximated by the int16 value round(x*128/ln2 +
# (127*128 - C)); one tensor_scalar (mult+add, f32->int16) per tile. C
# splits the round-vs-truncate convert difference; end-to-end softmax
# error is ~0.5% of the attention output, far inside the 2e-2 gate.
EXP_A = SCALE * 128.0 / np.log(2.0)
EXP_B = 127.0 * 128.0 - 7.0


def build_nc(stage=99):
    nc = bacc.Bacc("TRN2", target_bir_lowering=False, debug=False, num_devices=B)

    # ---- DRAM parameters (per core) ----
    x_d = nc.declare_dram_parameter("x", [C, T], F32R, isOutput=False)
    ident_d = nc.declare_dram_parameter("ident", [128, 128], F32R, isOutput=False)
    wqkT_d = nc.declare_dram_parameter("wqkT", [C, 2 * C], BF16, isOutput=False)
    wvT_d = nc.declare_dram_parameter("wvT", [C, C], BF16, isOutput=False)
    wpT_d = nc.declare_dram_parameter("wpT", [C, C], BF16, isOutput=False)
    gamma_d = nc.declare_dram_parameter("gamma", [C, 1], F32, isOutput=False)
    beta_d = nc.declare_dram_parameter("beta", [C, 1], F32, isOutput=False)
    bqk_d = nc.declare_dram_parameter("bqk", [2 * C, 1], F32, isOutput=False)
    bpe_d = nc.declare_dram_parameter("bpe", [C, 1], F32, isOutput=False)
    ind8_d = nc.declare_dram_parameter("ind8", [128, 8], F32, isOutput=False)
    indT8_d = nc.declare_dram_parameter("indT8", [8, 128], F32, isOutput=False)
    out_d = nc.declare_dram_parameter("out", [C, T], F32, isOutput=True)

    from contextlib import ExitStack

    with TileContext(nc) as tc, ExitStack() as sctx:
        pp = sctx.enter_context(tc.tile_pool(name="persist", bufs=1))
        qkp = sctx.enter_context(tc.tile_pool(name="qkpool", bufs=4))
        ep = sctx.enter_context(tc.tile_pool(name="epool", bufs=32))
        wp = sctx.enter_context(tc.tile_pool(name="workpool", bufs=2))
        # mm/small banks are used by GN + qk + vT only; they are closed and
        # recycled for the proj psums (scores/av pools stay open so the
        # pair-3 exp stream never blocks proj bank allocation).
        ps_scores = sctx.enter_context(tc.tile_pool(name="ps_scores", bufs=2, space="PSUM"))
        ps_av = sctx.enter_context(tc.tile_pool(name="ps_av", bufs=2, space="PSUM"))
        mm_ctx = ExitStack()
        ps_mm = mm_ctx.enter_context(tc.tile_pool(name="ps_mm", bufs=1, space="PSUM"))
        ps_small = mm_ctx.enter_context(tc.tile_pool(name="ps_small", bufs=1, space="PSUM"))
        if True:
            # ---- persistent sbuf tensors ----
            x_t = [pp.tile([128, T], F32R, name=f"x{i}", tag=f"x{i}") for i in range(NCT)]
            ident_t = pp.tile([128, 128], F32R, tag="ident")
            xn_t = [pp.tile([128, T], BF16, name=f"xn{i}", tag=f"xn{i}") for i in range(NCT)]
            wqkT_t = [pp.tile([128, 2 * C], BF16, name=f"wqkT{i}", tag=f"wqkT{i}") for i in range(NCT)]
            wvT_t = [pp.tile([128, C], BF16, name=f"wvT{i}", tag=f"wvT{i}") for i in range(NCT)]
            wpT_t = [pp.tile([128, C], BF16, name=f"wpT{i}", tag=f"wpT{i}") for i in range(NCT)]
            vT_t = [pp.tile([128, NH * (HD + 1)], BF16, name=f"vT{i}", tag=f"vT{i}") for i in range(NTT)]
            # a in [c, t] layout for proj: one tile, chunk i = channels
            # [128 i, 128 (i+1)); filled by the XBAR transposes.
            a_all = pp.tile([128, NCT, T], BF16, tag="a_all")
            # aT normalized staging, one per t-tile: [t within tile, 512 c]
            aTn_t = [pp.tile([128, C], BF16, name=f"aTn{i}", tag=f"aTn{i}") for i in range(NTT)]
            rz_t = [pp.tile([128, NP, 2], F32, name=f"rz{i}", tag=f"rz{i}") for i in range(NTT)]
            o_sb = [[pp.tile([128, 256], F32, name=f"o{ot}_{ch}", tag=f"o{ot}_{ch}")
                     for ch in range(4)] for ot in range(NCT)]
            gamma_t = pp.tile([128, NCT], F32, tag="gam")
            beta_t = pp.tile([128, NCT], F32, tag="bet")
            bqk_t = pp.tile([128, 2 * NCT], F32, tag="bqk")
            bpe_t = pp.tile([128, NCT], F32, tag="bpe")
            ind8_t = pp.tile([128, 8], F32, tag="ind8")
            indT8_t = pp.tile([8, 128], F32, tag="indT8")
            stats_t = pp.tile([128, 2 * NCT], F32, tag="stats")
            g8_t = pp.tile([8, 2 * NCT], F32, tag="g8")
            g2_t = pp.tile([8, NCT, 1], F32, tag="g2")
            scr_t = pp.tile([128, T], F32, tag="scr")

            # ---- input DMAs. Dispatch/transfer time serializes per issuing
            # engine, so alternate big tensors between the sync and gpsimd
            # queues in criticality order. GN-gating indicator matrices first.
            # x tiles first (they gate GN stats), spread over three queues;
            # wqkT interleaved right after (gates the first q/k matmuls),
            # then wvT / wpT in order of first use.
            nc.gpsimd.dma_start(out=ind8_t, in_=ind8_d.ap()[:, :])
            nc.sync.dma_start(out=x_t[0], in_=x_d.ap()[0:128, :])
            nc.gpsimd.dma_start(out=x_t[1], in_=x_d.ap()[128:256, :])
            nc.scalar.dma_start(out=x_t[2], in_=x_d.ap()[256:384, :])
            nc.gpsimd.dma_start(out=gamma_t, in_=gamma_d.ap().rearrange("(i p) one -> p (i one)", p=128))
            nc.gpsimd.dma_start(out=beta_t, in_=beta_d.ap().rearrange("(i p) one -> p (i one)", p=128))
            nc.sync.dma_start(out=x_t[3][:, 0:512], in_=x_d.ap()[384:512, 0:512])
            nc.scalar.dma_start(out=indT8_t, in_=indT8_d.ap()[:, :])
            nc.gpsimd.dma_start(out=x_t[3][:, 512:1024], in_=x_d.ap()[384:512, 512:1024])
            nc.sync.dma_start(out=wqkT_t[0], in_=wqkT_d.ap()[0:128, :])
            nc.gpsimd.dma_start(out=wqkT_t[1], in_=wqkT_d.ap()[128:256, :])
            nc.sync.dma_start(out=wqkT_t[2], in_=wqkT_d.ap()[256:384, :])
            nc.gpsimd.dma_start(out=wqkT_t[3], in_=wqkT_d.ap()[384:512, :])
            nc.scalar.dma_start(out=bqk_t, in_=bqk_d.ap().rearrange("(i p) one -> p (i one)", p=128))
            for i in range(NCT):
                eng = nc.sync if i % 2 == 0 else nc.gpsimd
                eng.dma_start(out=wvT_t[i], in_=wvT_d.ap()[i * 128:(i + 1) * 128, :])
            nc.gpsimd.dma_start(out=bpe_t, in_=bpe_d.ap().rearrange("(i p) one -> p (i one)", p=128))
            for i in range(NCT):
                eng = nc.sync if i % 2 == 0 else nc.gpsimd
                eng.dma_start(out=wpT_t[i], in_=wpT_d.ap()[i * 128:(i + 1) * 128, :])
            nc.scalar.dma_start(out=ident_t, in_=ident_d.ap()[:, :])

            # ================= GroupNorm =================
            # Each 16-channel group lives inside one 128-channel tile, so the
            # whole stats -> rstd -> xn chain runs per-tile: xn[i] completes
            # right after tile i's own square/sum, and the first q/k matmuls
            # start ~4us earlier than with a fused all-tile chain.
            zt = pp.tile([8, NCT, 1], F32, tag="zt")
            zq = pp.tile([8, NCT, 1], F32, tag="zq")
            gv = g8_t.rearrange("p (c two) -> p c two", two=2)
            for i in range(NCT):
                # per-channel sum (DVE) and sum-of-squares (ACT)
                nc.vector.reduce_sum(
                    out=stats_t[:, 2 * i:2 * i + 1], in_=x_t[i], axis=AX.X)
                nc.scalar.activation(out=scr_t, in_=x_t[i],
                                     func=AF.Square,
                                     accum_out=stats_t[:, 2 * i + 1:2 * i + 2])
                # sum the 16-partition groups of this tile
                g_ps = ps_small.tile([8, 2], F32, tag="misc")
                nc.tensor.matmul(out=g_ps, lhsT=ind8_t,
                                 rhs=stats_t[:, 2 * i:2 * i + 2], start=True, stop=True)
                nc.vector.tensor_scalar_mul(out=g8_t[:, 2 * i:2 * i + 2], in0=g_ps,
                                            scalar1=1.0 / NELEM_GROUP)
                gvi = gv[:, i:i + 1, :]
                zti = zt[:, i:i + 1, :]
                zqi = zq[:, i:i + 1, :]
                g2i = g2_t[:, i:i + 1, :]
                nc.vector.tensor_mul(g2i, gvi[:, :, 0:1], gvi[:, :, 0:1])
                nc.vector.tensor_sub(gvi[:, :, 1:2], gvi[:, :, 1:2], g2i)
                # rstd = rsqrt(var + eps), Newton from z0=1 entirely on DVE.
                vv = gvi[:, :, 1:2]
                nc.vector.tensor_scalar_add(out=vv, in0=vv, scalar1=EPS)
                # z1 = 1.5 - 0.5 v   (first Newton step from z0 = 1)
                nc.vector.tensor_scalar(out=zti, in0=vv, scalar1=-0.5, scalar2=1.5,
                                        op0=ALU.mult, op1=ALU.add)
                # z2 = z1 (1.5 - 0.5 v z1^2)
                nc.vector.tensor_mul(zqi, zti, zti)
                nc.vector.tensor_mul(zqi, zqi, vv)
                nc.vector.tensor_scalar(out=zqi, in0=zqi, scalar1=-0.5, scalar2=1.5,
                                        op0=ALU.mult, op1=ALU.add)
                nc.vector.tensor_mul(zti, zti, zqi)
                # z3 = z2 (1.5 - 0.5 v z2^2) -> write rstd into gvi[:, :, 1]
                nc.vector.tensor_mul(zqi, zti, zti)
                nc.vector.tensor_mul(zqi, zqi, vv)
                nc.vector.tensor_scalar(out=zqi, in0=zqi, scalar1=-0.5, scalar2=1.5,
                                        op0=ALU.mult, op1=ALU.add)
                nc.vector.tensor_mul(vv, zti, zqi)
                # broadcast (mean, rstd) to this tile's 128 channels
                mb_ps = ps_small.tile([128, 2], F32, tag="misc")
                nc.tensor.matmul(out=mb_ps, lhsT=indT8_t,
                                 rhs=g8_t[:, 2 * i:2 * i + 2], start=True, stop=True)
                scale_i = wp.tile([128, 1], F32, tag="scl")
                tmp_i = wp.tile([128, 1], F32, tag="tmpb")
                bias_i = wp.tile([128, 1], F32, tag="bia")
                nc.vector.tensor_mul(scale_i, gamma_t[:, i:i + 1], mb_ps[:, 1:2])
                nc.vector.tensor_mul(tmp_i, mb_ps[:, 0:1], scale_i)
                nc.vector.tensor_sub(bias_i, beta_t[:, i:i + 1], tmp_i)
                # xn tiles on three different engines so they finish in
                # parallel (they gate the first q/k matmuls)
                if i == 1:
                    nc.scalar.activation(out=xn_t[i], in_=x_t[i], func=AF.Identity,
                                         bias=bias_i, scale=scale_i)
                elif i == 2:
                    nc.gpsimd.tensor_scalar(
                        out=xn_t[i], in0=x_t[i], scalar1=scale_i, scalar2=bias_i,
                        op0=ALU.mult, op1=ALU.add)
                else:
                    nc.vector.tensor_scalar(
                        out=xn_t[i], in0=x_t[i], scalar1=scale_i, scalar2=bias_i,
                        op0=ALU.mult, op1=ALU.add)

            if stage == 0:
                for i in range(NCT):
                    nc.sync.dma_start(out=out_d.ap()[i * 128:(i + 1) * 128, :].bitcast(BF16)[:, :T], in_=xn_t[i])

            # Fold the (proj bias + Wp@b_v) into x in place: the residual
            # epilogue is then an identity-matmul accumulate. Runs after xn
            # (so GN sees the raw x) on the otherwise-idle Pool engine.
            for i in range(NCT):
                nc.gpsimd.tensor_scalar_add(out=x_t[i], in0=x_t[i],
                                            scalar1=bpe_t[:, i:i + 1])

            # ================= attention (head pairs) + interleaved q/k =====
            def emit_qk(p):
                # q/k channel tiles for pair p: qkv rows p*128 (q), C+p*128 (k).
                q_tile = qkp.tile([128, T], BF16, name=f"q{p}", tag="qk")
                k_tile = qkp.tile([128, T], BF16, name=f"k{p}", tag="qk")
                gi = 0
                for nh in range(2):
                    for mt, dstt in ((p, q_tile), (NCT + p, k_tile)):
                        if gi % 2 == 0:
                            acc = ps_mm.tile([128, 512], F32, tag="mm")
                        else:
                            acc = ps_small.tile([128, 512], F32, tag="misc")
                        gi += 1
                        for kc in range(NCT):
                            nc.tensor.matmul(
                                out=acc,
                                lhsT=wqkT_t[kc][:, mt * 128:(mt + 1) * 128],
                                rhs=xn_t[kc][:, nh * 512:(nh + 1) * 512],
                                start=(kc == 0), stop=(kc == NCT - 1))
                        nc.vector.tensor_scalar_add(
                            out=dstt[:, nh * 512:(nh + 1) * 512], in0=acc,
                            scalar1=bqk_t[:, mt:mt + 1])
                return q_tile, k_tile

            npairs = NP if stage >= 1 else 0

            def emit_exp(sps, width, on_dve):
                et = ep.tile([128, width], BF16, tag="E")
                if on_dve:
                    nc.vector.tensor_scalar(
                        out=et.bitcast(mybir.dt.int16), in0=sps,
                        scalar1=EXP_A, scalar2=EXP_B,
                        op0=ALU.mult, op1=ALU.add)
                else:
                    nc.scalar.activation(out=et, in_=sps, func=AF.Exp, scale=SCALE)
                return et

            # (sc, j) score tiles whose exp runs on DVE (Schraudolph) instead
            # of ACT: the exp stream is the kernel bottleneck and DVE has
            # slack. For the last pair only early (th=0) tiles are offloaded
            # so the DVE tail (avT normalize) stays clean.
            DVE_EXP = {(1, 1), (3, 1), (5, 1), (7, 1)}
            DVE_EXP_LAST = {(0, 1), (2, 1), (3, 1), (4, 1), (5, 1), (6, 1)}

            def emit_scores_exp(p):
                # E tiles for pair p. Returns e[(sc, j)] -> tile [128, T], or
                # for the last pair e[(sc, j)] -> (tile_th0, tile_th1) halves
                # so the tail avT/proj work overlaps the th=1 exp stream.
                q_tile, k_tile = qk_tiles[p]
                last = p == NP - 1
                e_tiles = {}
                if not last:
                    for sc in range(NTT):
                        for j in range(2):
                            sps = ps_scores.tile([128, 1024], F32, tag="scores")
                            for th in range(2):
                                nc.tensor.matmul(
                                    out=sps[:, th * 512:(th + 1) * 512],
                                    lhsT=k_tile[j * 64:(j + 1) * 64, sc * 128:(sc + 1) * 128],
                                    rhs=q_tile[j * 64:(j + 1) * 64, th * 512:(th + 1) * 512],
                                    start=True, stop=True)
                            e_tiles[(sc, j)] = emit_exp(
                                sps, 1024, (sc, j) in DVE_EXP)
                else:
                    halves = {}
                    for th in range(2):
                        for sc in range(NTT):
                            for j in range(2):
                                sps = ps_scores.tile([128, 512], F32, tag="scores")
                                nc.tensor.matmul(
                                    out=sps,
                                    lhsT=k_tile[j * 64:(j + 1) * 64, sc * 128:(sc + 1) * 128],
                                    rhs=q_tile[j * 64:(j + 1) * 64, th * 512:(th + 1) * 512],
                                    start=True, stop=True)
                                halves[(sc, j, th)] = emit_exp(
                                    sps, 512,
                                    th == 0 and (sc, j) in DVE_EXP_LAST)
                    for sc in range(NTT):
                        for j in range(2):
                            e_tiles[(sc, j)] = (halves[(sc, j, 0)], halves[(sc, j, 1)])
                return e_tiles

            def emit_vt():
                # vT = xn^T @ WvT (+ ones cols); fills pair-0 exp gaps on PE
                for tt in range(NTT):
                    if tt % 2 == 0:
                        acc = ps_mm.tile([128, C], F32, tag="mm")
                    else:
                        acc = ps_small.tile([128, C], F32, tag="misc")
                    for kc in range(NCT):
                        nc.tensor.matmul(
                            out=acc,
                            lhsT=xn_t[kc][:, tt * 128:(tt + 1) * 128],
                            rhs=wvT_t[kc],
                            start=(kc == 0), stop=(kc == NCT - 1))
                    nc.gpsimd.memset(vT_t[tt], 1.0)
                    vdst = vT_t[tt].rearrange("p (h x) -> p h x", x=HD + 1)
                    vsrc = acc.rearrange("p (h x) -> p h x", x=HD)
                    nc.vector.tensor_copy(vdst[:, :, 0:HD], vsrc)

            def e_slice(e_entry, tt):
                # lhsT slice [128 s, 128 t] of pair-p E for t-tile tt
                if isinstance(e_entry, tuple):
                    th = tt // 4
                    return e_entry[th][:, (tt % 4) * 128:(tt % 4 + 1) * 128]
                return e_entry[:, tt * 128:(tt + 1) * 128]

            def emit_avt_tile(p, e_tiles, tt):
                # avT psum [128 t, 2*65]: col j*65+64 = Z_j (vT ones col).
                # The tile is padded to a full PSUM bank and the 16 matmuls
                # form ONE accumulation group: start=True zero-marks the
                # whole 2KB bank, so a second start inside the same bank
                # would wipe the other head's partial sums.
                aps = ps_av.tile([128, 2 * (HD + 1)], F32, tag="av",
                                 padded_shape=(128, 512))
                for sc in range(NTT):
                    for j in range(2):
                        h = 2 * p + j
                        nc.tensor.matmul(
                            out=aps[:, j * 65:(j + 1) * 65],
                            lhsT=e_slice(e_tiles[(sc, j)], tt),
                            rhs=vT_t[sc][:, h * 65:(h + 1) * 65],
                            start=(sc == 0 and j == 0),
                            stop=(sc == NTT - 1 and j == 1),
                            skip_group_check=True)
                # rz = 1/Z for both heads of the pair
                apv = aps.rearrange("p (j x) -> p j x", x=HD + 1)
                nc.vector.reciprocal(out=rz_t[tt][:, p, :],
                                     in_=apv[:, :, HD])
                # normalize + downcast into the aT staging tile
                for j in range(2):
                    nc.vector.tensor_scalar_mul(
                        out=aTn_t[tt][:, p * 128 + j * 64:p * 128 + (j + 1) * 64],
                        in0=apv[:, j, 0:HD],
                        scalar1=rz_t[tt][:, p, j:j + 1])

            def emit_avt(p, e_tiles):
                last = p == NP - 1
                for tt in range(NTT):
                    emit_avt_tile(p, e_tiles, tt)
                    if last:
                        # all pairs' aTn for this t-tile are now complete:
                        # XBAR-transpose [128 t, 512 c] -> a_all[:, :, t-tile]
                        nc.sync.dma_start_transpose(
                            out=a_all[:, :, tt * 128:(tt + 1) * 128],
                            in_=aTn_t[tt])

            # software pipeline: scores/exp(p) -> qk(p+1) -> avT(p-1).
            qk_tiles = {0: emit_qk(0)} if npairs else {}
            e_store = {}
            if stage == 1 and npairs:
                q_tile, k_tile = qk_tiles[0]
                nc.sync.dma_start(out=out_d.ap()[0:128, :].bitcast(BF16)[:, :T], in_=q_tile)
                nc.sync.dma_start(out=out_d.ap()[128:256, :].bitcast(BF16)[:, :T], in_=k_tile)
            elif npairs:
                for p in range(npairs):
                    e_store[p] = emit_scores_exp(p)
                    if p + 1 < npairs:
                        qk_tiles[p + 1] = emit_qk(p + 1)
                    if p == 0:
                        emit_vt()
                    if p >= 1:
                        emit_avt(p - 1, e_store.pop(p - 1))
                emit_avt(npairs - 1, e_store.pop(npairs - 1))

        if stage == 2:
            for i in range(NCT):
                nc.sync.dma_start(out=out_d.ap()[i * 128:(i + 1) * 128, :].bitcast(BF16)[:, :T],
                                  in_=a_all[:, i, :])

        # ================= proj + residual =================
        # The residual (x + bpe, pre-folded into x_t) is accumulated into the
        # proj psum by one extra identity matmul, and the psum is DMA'd to
        # DRAM directly: no DVE work in the tail, so the tail dependency
        # chain is norm(DVE) -> transpose(DMA) -> proj(PE) -> out(DMA).
        # 256-col chunks start as soon as their two t-tile transposes land.
        mm_ctx.close()  # recycle the qk/vT mm banks for the proj pool
        with (
            tc.tile_pool(name="ps_proj", bufs=2, space="PSUM") as ps_proj,
        ):
            for ch in range(4 if stage >= 3 else 0):
                for ot in range(NCT):
                    # pad to a full PSUM bank: pool slots are packed without
                    # bank alignment, and two accumulation groups sharing a
                    # bank clobber each other (start zero-marks the bank)
                    acc = ps_proj.tile([128, 256], F32, tag="proj",
                                       padded_shape=(128, 512))
                    for kc in range(NCT):
                        nc.tensor.matmul(
                            out=acc,
                            lhsT=wpT_t[kc][:, ot * 128:(ot + 1) * 128],
                            rhs=a_all[:, kc, ch * 256:(ch + 1) * 256],
                            start=(kc == 0), stop=False)
                    nc.tensor.matmul(
                        out=acc, lhsT=ident_t,
                        rhs=x_t[ot][:, ch * 256:(ch + 1) * 256],
                        start=False, stop=True)
                    # drain psum -> sbuf on DVE early (ACT still busy with
                    # exps), on the freed-up ACT for the late chunks
                    if ch < 2:
                        nc.vector.tensor_copy(o_sb[ot][ch], acc)
                    else:
                        nc.scalar.copy(o_sb[ot][ch], acc)
                    oeng = nc.sync if ot % 2 == 0 else nc.gpsimd
                    oeng.dma_start(
                        out=out_d.ap()[ot * 128:(ot + 1) * 128, ch * 256:(ch + 1) * 256],
                        in_=o_sb[ot][ch])

    nc.finalize()
    return nc


def make_in_maps(x, gn_gamma, gn_beta, w_qkv, b_qkv, w_proj, b_proj):
    x = np.asarray(x, np.float32)
    w_qkv = np.asarray(w_qkv, np.float32)
    b_qkv = np.asarray(b_qkv, np.float32)
    w_proj = np.asarray(w_proj, np.float32)
    b_proj = np.asarray(b_proj, np.float32)

    bf = ml_dtypes.bfloat16
    wqkT = np.ascontiguousarray(w_qkv[:2 * C].T).astype(bf)      # [C, 2C]
    wvT = np.ascontiguousarray(w_qkv[2 * C:].T).astype(bf)       # [C, C]
    wpT = np.ascontiguousarray(w_proj.T).astype(bf)              # [C, C]
    bqk = np.ascontiguousarray(b_qkv[:2 * C]).reshape(2 * C, 1)
    bv = b_qkv[2 * C:]
    bpe = (b_proj + w_proj @ bv).reshape(C, 1).astype(np.float32)
    gamma = np.asarray(gn_gamma, np.float32).reshape(C, 1)
    beta = np.asarray(gn_beta, np.float32).reshape(C, 1)

    pidx = np.arange(128)
    ind8 = (pidx[:, None] // GSZ == np.arange(8)[None, :]).astype(np.float32)
    indT8 = np.ascontiguousarray(ind8.T)

    shared = {
        "wqkT": wqkT, "wvT": wvT, "wpT": wpT,
        "gamma": gamma, "beta": beta, "bqk": bqk, "bpe": np.ascontiguousarray(bpe),
        "ind8": ind8, "indT8": indT8, "ident": np.eye(128, dtype=np.float32),
    }
    xf = x.reshape(B, C, T)
    return [dict(shared, x=np.ascontiguousarray(xf[b])) for b in range(B)]


_NC_CACHE = None


def kernel(x, gn_gamma, gn_beta, w_qkv, b_qkv, w_proj, b_proj):
    global _NC_CACHE
    if _NC_CACHE is None:
        _NC_CACHE = build_nc()
    in_maps = make_in_maps(x, gn_gamma, gn_beta, w_qkv, b_qkv, w_proj, b_proj)
    res = run_bass_kernel_spmd(_NC_CACHE, in_maps, core_ids=list(range(B)))
    out = np.stack([res.results[b]["out"] for b in range(B)])
    return out.reshape(B, C, H, W).astype(np.float32)


# revision 29
# speedup vs baseline: 1.0270x; 1.0270x over previous
"""Trainium2 Bass kernel for an AttentionBlock (GroupNorm + MHSA + proj + residual).

Problem shapes (hardcoded): x [B=8, C=512, H=32, W=32], T = H*W = 1024,
NH=8 heads (head_dim 64), GroupNorm groups G=32, eps 1e-5.

Sharding: data-parallel over batch B across the 8 NeuronCores — one batch
element per core, no collectives.

Per-core dataflow (all layouts [partition, free]):
  x        [C, T]   4 sbuf tiles of [128, 1024] f32
  GroupNorm stats: per-tile row sums (DVE) / sums-of-squares (ACT Square with
           accum_out), group-summed across partitions with a tiny indicator
           matmul, rstd via Newton rsqrt on DVE, then per-channel scale/bias
           broadcast back with another tiny matmul.
  xn       [C, T]   = x*scale + bias (DVE tensor_scalar), bf16
  q,k = W_qk^T.T @ xn   -> bf16 tiles [128, 1024]
  vT  = xn.T @ WvT      -> vT tiles [128, 8*65] bf16 (col 64 of each head
                           block memset to 1.0: fused softmax-denominator)
  scoresT[s,t] = k_h^T q_h : K=64 matmuls, two heads (j) per pair tile.
  E = exp(scoresT/8)    -> bf16 sbuf (one ACT pass per [128, 1024] psum tile;
                           the last pair is split into t-halves so the tail
                           av/proj work overlaps the remaining exp stream)
  avT: out[t, j*65+d] = sum_s E_j[s,t] vT[s, h*65+d]  (psum [128, 130] per
                           (pair, t-tile); column 64 of each j-block is the
                           softmax denominator Z because of the vT ones col)
  normalize: rz = 1/Z (DVE reciprocal of the two Z columns), then per-j
                           tensor_scalar multiply psum -> aTn bf16 [t, c]
  transpose: XBAR dma_start_transpose per t-tile [128 t, 512 c] ->
                           a_all[p, pair, t] (the [c, t] layout proj needs)
  out = WpT.T @ a + x + bpe  (bpe folded into x mid-kernel on DVE; epilogue
                           is one DVE add psum+x) -> DMA out [C, T]
"""

import numpy as np
import ml_dtypes

import concourse.bacc as bacc
from concourse import mybir
from concourse.tile import TileContext
from concourse.bass_utils import run_bass_kernel_spmd

F32 = mybir.dt.float32
F32R = mybir.dt.float32r
BF16 = mybir.dt.bfloat16
AF = mybir.ActivationFunctionType
ALU = mybir.AluOpType
AX = mybir.AxisListType

B = 8
C = 512
H = W = 32
T = H * W            # 1024
NH = 8
HD = C // NH         # 64
G = 32               # groupnorm groups
GSZ = C // G         # 16 channels per group
EPS = 1e-5
NCT = C // 128       # 4 channel tiles
NTT = T // 128       # 8 token tiles
NP = NH // 2         # 4 head pairs
SCALE = 1.0 / np.sqrt(HD)   # 0.125
NELEM_GROUP = GSZ * T       # 16384 elements per group

# Schraudolph-style exp for the DVE-offloaded score tiles: the bf16 bit
# pattern of exp(x) is approximated by the int16 value round(x*128/ln2 +
# (127*128 - C)); one tensor_scalar (mult+add, f32->int16) per tile. C
# splits the round-vs-truncate convert difference; end-to-end softmax
# error is ~0.5% of the attention output, far inside the 2e-2 gate.
EXP_A = SCALE * 128.0 / np.log(2.0)
EXP_B = 127.0 * 128.0 - 7.0


def build_nc(stage=99):
    nc = bacc.Bacc("TRN2", target_bir_lowering=False, debug=False, num_devices=B)

    # ---- DRAM parameters (per core) ----
    x_d = nc.declare_dram_parameter("x", [C, T], F32R, isOutput=False)
    ident_d = nc.declare_dram_parameter("ident", [128, 128], F32R, isOutput=False)
    wqkT_d = nc.declare_dram_parameter("wqkT", [C, 2 * C], BF16, isOutput=False)
    wvT_d = nc.declare_dram_parameter("wvT", [C, C], BF16, isOutput=False)
    wpT_d = nc.declare_dram_parameter("wpT", [C, C], BF16, isOutput=False)
    gamma_d = nc.declare_dram_parameter("gamma", [C, 1], F32, isOutput=False)
    beta_d = nc.declare_dram_parameter("beta", [C, 1], F32, isOutput=False)
    bqk_d = nc.declare_dram_parameter("bqk", [2 * C, 1], F32, isOutput=False)
    bpe_d = nc.declare_dram_parameter("bpe", [C, 1], F32, isOutput=False)
    ind8_d = nc.declare_dram_parameter("ind8", [128, 8], F32, isOutput=False)
    indT8_d = nc.declare_dram_parameter("indT8", [8, 128], F32, isOutput=False)
    out_d = nc.declare_dram_parameter("out", [C, T], F32, isOutput=True)

    from contextlib import ExitStack

    with TileContext(nc) as tc, ExitStack() as sctx:
        pp = sctx.enter_context(tc.tile_pool(name="persist", bufs=1))
        qkp = sctx.enter_context(tc.tile_pool(name="qkpool", bufs=4))
        ep = sctx.enter_context(tc.tile_pool(name="epool", bufs=32))
        wp = sctx.enter_context(tc.tile_pool(name="workpool", bufs=2))
        # mm/small banks are used by GN + qk + vT only; they are closed and
        # recycled for the proj psums (scores/av pools stay open so the
        # pair-3 exp stream never blocks proj bank allocation).
        ps_scores = sctx.enter_context(tc.tile_pool(name="ps_scores", bufs=2, space="PSUM"))
        ps_av = sctx.enter_context(tc.tile_pool(name="ps_av", bufs=2, space="PSUM"))
        mm_ctx = ExitStack()
        ps_mm = mm_ctx.enter_context(tc.tile_pool(name="ps_mm", bufs=1, space="PSUM"))
        ps_small = mm_ctx.enter_context(tc.tile_pool(name="ps_small", bufs=1, space="PSUM"))
        if True:
            # ---- persistent sbuf tensors ----
            x_t = [pp.tile([128, T], F32R, name=f"x{i}", tag=f"x{i}") for i in range(NCT)]
            ident_t = pp.tile([128, 128], F32R, tag="ident")
            xn_t = [pp.tile([128, T], BF16, name=f"xn{i}", tag=f"xn{i}") for i in range(NCT)]
            wqkT_t = [pp.tile([128, 2 * C], BF16, name=f"wqkT{i}", tag=f"wqkT{i}") for i in range(NCT)]
            wvT_t = [pp.tile([128, C], BF16, name=f"wvT{i}", tag=f"wvT{i}") for i in range(NCT)]
            wpT_t = [pp.tile([128, C], BF16, name=f"wpT{i}", tag=f"wpT{i}") for i in range(NCT)]
            vT_t = [pp.tile([128, NH * (HD + 1)], BF16, name=f"vT{i}", tag=f"vT{i}") for i in range(NTT)]
            # a in [c, t] layout for proj: one tile, chunk i = channels
            # [128 i, 128 (i+1)); filled by the XBAR transposes.
            a_all = pp.tile([128, NCT, T], BF16, tag="a_all")
            # aT normalized staging, one per t-tile: [t within tile, 512 c]
            aTn_t = [pp.tile([128, C], BF16, name=f"aTn{i}", tag=f"aTn{i}") for i in range(NTT)]
            rz_t = [pp.tile([128, NP, 2], F32, name=f"rz{i}", tag=f"rz{i}") for i in range(NTT)]
            o_sb = [[pp.tile([128, 256], F32, name=f"o{ot}_{ch}", tag=f"o{ot}_{ch}")
                     for ch in range(4)] for ot in range(NCT)]
            gamma_t = pp.tile([128, NCT], F32, tag="gam")
            beta_t = pp.tile([128, NCT], F32, tag="bet")
            bqk_t = pp.tile([128, 2 * NCT], F32, tag="bqk")
            bpe_t = pp.tile([128, NCT], F32, tag="bpe")
            ind8_t = pp.tile([128, 8], F32, tag="ind8")
            indT8_t = pp.tile([8, 128], F32, tag="indT8")
            stats_t = pp.tile([128, 2 * NCT], F32, tag="stats")
            g8_t = pp.tile([8, 2 * NCT], F32, tag="g8")
            g2_t = pp.tile([8, NCT, 1], F32, tag="g2")
            scr_t = pp.tile([128, T], F32, tag="scr")

            # ---- input DMAs. Dispatch/transfer time serializes per issuing
            # engine, so alternate big tensors between the sync and gpsimd
            # queues in criticality order. GN-gating indicator matrices first.
            # x tiles first (they gate GN stats), spread over three queues;
            # wqkT interleaved right after (gates the first q/k matmuls),
            # then wvT / wpT in order of first use.
            nc.gpsimd.dma_start(out=ind8_t, in_=ind8_d.ap()[:, :])
            nc.sync.dma_start(out=x_t[0], in_=x_d.ap()[0:128, :])
            nc.gpsimd.dma_start(out=x_t[1], in_=x_d.ap()[128:256, :])
            nc.scalar.dma_start(out=x_t[2], in_=x_d.ap()[256:384, :])
            nc.gpsimd.dma_start(out=gamma_t, in_=gamma_d.ap().rearrange("(i p) one -> p (i one)", p=128))
            nc.gpsimd.dma_start(out=beta_t, in_=beta_d.ap().rearrange("(i p) one -> p (i one)", p=128))
            nc.sync.dma_start(out=x_t[3][:, 0:512], in_=x_d.ap()[384:512, 0:512])
            nc.gpsimd.dma_start(out=indT8_t, in_=indT8_d.ap()[:, :])
            nc.gpsimd.dma_start(out=x_t[3][:, 512:1024], in_=x_d.ap()[384:512, 512:1024])
            nc.sync.dma_start(out=wqkT_t[0], in_=wqkT_d.ap()[0:128, :])
            nc.gpsimd.dma_start(out=wqkT_t[1], in_=wqkT_d.ap()[128:256, :])
            nc.sync.dma_start(out=wqkT_t[2], in_=wqkT_d.ap()[256:384, :])
            nc.gpsimd.dma_start(out=wqkT_t[3], in_=wqkT_d.ap()[384:512, :])
            nc.gpsimd.dma_start(out=bqk_t, in_=bqk_d.ap().rearrange("(i p) one -> p (i one)", p=128))
            for i in range(NCT):
                eng = nc.sync if i % 2 == 0 else nc.gpsimd
                eng.dma_start(out=wvT_t[i], in_=wvT_d.ap()[i * 128:(i + 1) * 128, :])
            nc.gpsimd.dma_start(out=bpe_t, in_=bpe_d.ap().rearrange("(i p) one -> p (i one)", p=128))
            for i in range(NCT):
                eng = nc.sync if i % 2 == 0 else nc.gpsimd
                eng.dma_start(out=wpT_t[i], in_=wpT_d.ap()[i * 128:(i + 1) * 128, :])
            nc.gpsimd.dma_start(out=ident_t, in_=ident_d.ap()[:, :])

            # ================= GroupNorm =================
            # Each 16-channel group lives inside one 128-channel tile, so the
            # whole stats -> rstd -> xn chain runs per-tile: xn[i] completes
            # right after tile i's own square/sum, and the first q/k matmuls
            # start ~4us earlier than with a fused all-tile chain.
            zt = pp.tile([8, NCT, 1], F32, tag="zt")
            zq = pp.tile([8, NCT, 1], F32, tag="zq")
            gv = g8_t.rearrange("p (c two) -> p c two", two=2)
            for i in range(NCT):
                # per-channel sum (DVE) and sum-of-squares (ACT)
                nc.vector.reduce_sum(
                    out=stats_t[:, 2 * i:2 * i + 1], in_=x_t[i], axis=AX.X)
                nc.scalar.activation(out=scr_t, in_=x_t[i],
                                     func=AF.Square,
                                     accum_out=stats_t[:, 2 * i + 1:2 * i + 2])
                # sum the 16-partition groups of this tile
                g_ps = ps_small.tile([8, 2], F32, tag="misc")
                nc.tensor.matmul(out=g_ps, lhsT=ind8_t,
                                 rhs=stats_t[:, 2 * i:2 * i + 2], start=True, stop=True)
                nc.vector.tensor_scalar_mul(out=g8_t[:, 2 * i:2 * i + 2], in0=g_ps,
                                            scalar1=1.0 / NELEM_GROUP)
                gvi = gv[:, i:i + 1, :]
                zti = zt[:, i:i + 1, :]
                zqi = zq[:, i:i + 1, :]
                g2i = g2_t[:, i:i + 1, :]
                nc.vector.tensor_mul(g2i, gvi[:, :, 0:1], gvi[:, :, 0:1])
                nc.vector.tensor_sub(gvi[:, :, 1:2], gvi[:, :, 1:2], g2i)
                # rstd = rsqrt(var + eps), Newton from z0=1 entirely on DVE.
                vv = gvi[:, :, 1:2]
                nc.vector.tensor_scalar_add(out=vv, in0=vv, scalar1=EPS)
                # z1 = 1.5 - 0.5 v   (first Newton step from z0 = 1)
                nc.vector.tensor_scalar(out=zti, in0=vv, scalar1=-0.5, scalar2=1.5,
                                        op0=ALU.mult, op1=ALU.add)
                # z2 = z1 (1.5 - 0.5 v z1^2)
                nc.vector.tensor_mul(zqi, zti, zti)
                nc.vector.tensor_mul(zqi, zqi, vv)
                nc.vector.tensor_scalar(out=zqi, in0=zqi, scalar1=-0.5, scalar2=1.5,
                                        op0=ALU.mult, op1=ALU.add)
                nc.vector.tensor_mul(zti, zti, zqi)
                # z3 = z2 (1.5 - 0.5 v z2^2) -> write rstd into gvi[:, :, 1]
                nc.vector.tensor_mul(zqi, zti, zti)
                nc.vector.tensor_mul(zqi, zqi, vv)
                nc.vector.tensor_scalar(out=zqi, in0=zqi, scalar1=-0.5, scalar2=1.5,
                                        op0=ALU.mult, op1=ALU.add)
                nc.vector.tensor_mul(vv, zti, zqi)
                # broadcast (mean, rstd) to this tile's 128 channels
                mb_ps = ps_small.tile([128, 2], F32, tag="misc")
                nc.tensor.matmul(out=mb_ps, lhsT=indT8_t,
                                 rhs=g8_t[:, 2 * i:2 * i + 2], start=True, stop=True)
                scale_i = wp.tile([128, 1], F32, tag="scl")
                tmp_i = wp.tile([128, 1], F32, tag="tmpb")
                bias_i = wp.tile([128, 1], F32, tag="bia")
                nc.vector.tensor_mul(scale_i, gamma_t[:, i:i + 1], mb_ps[:, 1:2])
                nc.vector.tensor_mul(tmp_i, mb_ps[:, 0:1], scale_i)
                nc.vector.tensor_sub(bias_i, beta_t[:, i:i + 1], tmp_i)
                # xn tiles on three different engines so they finish in
                # parallel (they gate the first q/k matmuls)
                if i == 1:
                    nc.scalar.activation(out=xn_t[i], in_=x_t[i], func=AF.Identity,
                                         bias=bias_i, scale=scale_i)
                else:
                    nc.vector.tensor_scalar(
                        out=xn_t[i], in0=x_t[i], scalar1=scale_i, scalar2=bias_i,
                        op0=ALU.mult, op1=ALU.add)

            if stage == 0:
                for i in range(NCT):
                    nc.sync.dma_start(out=out_d.ap()[i * 128:(i + 1) * 128, :].bitcast(BF16)[:, :T], in_=xn_t[i])

            # Fold the (proj bias + Wp@b_v) into x in place: the residual
            # epilogue is then an identity-matmul accumulate. Runs after xn
            # (so GN sees the raw x) on the otherwise-idle Pool engine.
            for i in range(NCT):
                nc.gpsimd.tensor_scalar_add(out=x_t[i], in0=x_t[i],
                                            scalar1=bpe_t[:, i:i + 1])

            # ================= attention (head pairs) + interleaved q/k =====
            def emit_qk(p):
                # q/k channel tiles for pair p: qkv rows p*128 (q), C+p*128 (k).
                q_tile = qkp.tile([128, T], BF16, name=f"q{p}", tag="qk")
                k_tile = qkp.tile([128, T], BF16, name=f"k{p}", tag="qk")
                gi = 0
                for nh in range(2):
                    for mt, dstt in ((p, q_tile), (NCT + p, k_tile)):
                        if gi % 2 == 0:
                            acc = ps_mm.tile([128, 512], F32, tag="mm")
                        else:
                            acc = ps_small.tile([128, 512], F32, tag="misc")
                        gi += 1
                        for kc in range(NCT):
                            nc.tensor.matmul(
                                out=acc,
                                lhsT=wqkT_t[kc][:, mt * 128:(mt + 1) * 128],
                                rhs=xn_t[kc][:, nh * 512:(nh + 1) * 512],
                                start=(kc == 0), stop=(kc == NCT - 1))
                        nc.vector.tensor_scalar_add(
                            out=dstt[:, nh * 512:(nh + 1) * 512], in0=acc,
                            scalar1=bqk_t[:, mt:mt + 1])
                return q_tile, k_tile

            npairs = NP if stage >= 1 else 0

            def emit_exp(sps, width, on_dve):
                et = ep.tile([128, width], BF16, tag="E")
                if on_dve:
                    nc.vector.tensor_scalar(
                        out=et.bitcast(mybir.dt.int16), in0=sps,
                        scalar1=EXP_A, scalar2=EXP_B,
                        op0=ALU.mult, op1=ALU.add)
                else:
                    nc.scalar.activation(out=et, in_=sps, func=AF.Exp, scale=SCALE)
                return et

            # (sc, j) score tiles whose exp runs on DVE (Schraudolph) instead
            # of ACT: the exp stream is the kernel bottleneck and DVE has
            # slack. For the last pair only early (th=0) tiles are offloaded
            # so the DVE tail (avT normalize) stays clean.
            DVE_EXP = {(1, 1), (3, 1), (5, 1), (7, 1)}
            DVE_EXP_LAST = {(0, 1), (2, 1), (3, 1), (4, 1), (5, 1), (6, 1)}

            def emit_scores_exp(p):
                # E tiles for pair p. Returns e[(sc, j)] -> tile [128, T], or
                # for the last pair e[(sc, j)] -> (tile_th0, tile_th1) halves
                # so the tail avT/proj work overlaps the th=1 exp stream.
                q_tile, k_tile = qk_tiles[p]
                last = p == NP - 1
                e_tiles = {}
                if not last:
                    for sc in range(NTT):
                        for j in range(2):
                            sps = ps_scores.tile([128, 1024], F32, tag="scores")
                            for th in range(2):
                                nc.tensor.matmul(
                                    out=sps[:, th * 512:(th + 1) * 512],
                                    lhsT=k_tile[j * 64:(j + 1) * 64, sc * 128:(sc + 1) * 128],
                                    rhs=q_tile[j * 64:(j + 1) * 64, th * 512:(th + 1) * 512],
                                    start=True, stop=True)
                            e_tiles[(sc, j)] = emit_exp(
                                sps, 1024, (sc, j) in DVE_EXP)
                else:
                    halves = {}
                    for th in range(2):
                        for sc in range(NTT):
                            for j in range(2):
                                sps = ps_scores.tile([128, 512], F32, tag="scores")
                                nc.tensor.matmul(
                                    out=sps,
                                    lhsT=k_tile[j * 64:(j + 1) * 64, sc * 128:(sc + 1) * 128],
                                    rhs=q_tile[j * 64:(j + 1) * 64, th * 512:(th + 1) * 512],
                                    start=True, stop=True)
                                halves[(sc, j, th)] = emit_exp(
                                    sps, 512,
                                    th == 0 and (sc, j) in DVE_EXP_LAST)
                    for sc in range(NTT):
                        for j in range(2):
                            e_tiles[(sc, j)] = (halves[(sc, j, 0)], halves[(sc, j, 1)])
                return e_tiles

            def emit_vt():
                # vT = xn^T @ WvT (+ ones cols); fills pair-0 exp gaps on PE
                for tt in range(NTT):
                    if tt % 2 == 0:
                        acc = ps_mm.tile([128, C], F32, tag="mm")
                    else:
                        acc = ps_small.tile([128, C], F32, tag="misc")
                    for kc in range(NCT):
                        nc.tensor.matmul(
                            out=acc,
                            lhsT=xn_t[kc][:, tt * 128:(tt + 1) * 128],
                            rhs=wvT_t[kc],
                            start=(kc == 0), stop=(kc == NCT - 1))
                    nc.gpsimd.memset(vT_t[tt], 1.0)
                    vdst = vT_t[tt].rearrange("p (h x) -> p h x", x=HD + 1)
                    vsrc = acc.rearrange("p (h x) -> p h x", x=HD)
                    nc.vector.tensor_copy(vdst[:, :, 0:HD], vsrc)

            def e_slice(e_entry, tt):
                # lhsT slice [128 s, 128 t] of pair-p E for t-tile tt
                if isinstance(e_entry, tuple):
                    th = tt // 4
                    return e_entry[th][:, (tt % 4) * 128:(tt % 4 + 1) * 128]
                return e_entry[:, tt * 128:(tt + 1) * 128]

            def emit_avt_tile(p, e_tiles, tt):
                # avT psum [128 t, 2*65]: col j*65+64 = Z_j (vT ones col).
                # The tile is padded to a full PSUM bank and the 16 matmuls
                # form ONE accumulation group: start=True zero-marks the
                # whole 2KB bank, so a second start inside the same bank
                # would wipe the other head's partial sums.
                aps = ps_av.tile([128, 2 * (HD + 1)], F32, tag="av",
                                 padded_shape=(128, 512))
                for sc in range(NTT):
                    for j in range(2):
                        h = 2 * p + j
                        nc.tensor.matmul(
                            out=aps[:, j * 65:(j + 1) * 65],
                            lhsT=e_slice(e_tiles[(sc, j)], tt),
                            rhs=vT_t[sc][:, h * 65:(h + 1) * 65],
                            start=(sc == 0 and j == 0),
                            stop=(sc == NTT - 1 and j == 1),
                            skip_group_check=True)
                # rz = 1/Z for both heads of the pair
                apv = aps.rearrange("p (j x) -> p j x", x=HD + 1)
                nc.vector.reciprocal(out=rz_t[tt][:, p, :],
                                     in_=apv[:, :, HD])
                # normalize + downcast into the aT staging tile
                for j in range(2):
                    nc.vector.tensor_scalar_mul(
                        out=aTn_t[tt][:, p * 128 + j * 64:p * 128 + (j + 1) * 64],
                        in0=apv[:, j, 0:HD],
                        scalar1=rz_t[tt][:, p, j:j + 1])

            def emit_avt(p, e_tiles):
                last = p == NP - 1
                for tt in range(NTT):
                    emit_avt_tile(p, e_tiles, tt)
                    if last:
                        # all pairs' aTn for this t-tile are now complete:
                        # XBAR-transpose [128 t, 512 c] -> a_all[:, :, t-tile]
                        nc.sync.dma_start_transpose(
                            out=a_all[:, :, tt * 128:(tt + 1) * 128],
                            in_=aTn_t[tt])

            # software pipeline: scores/exp(p) -> qk(p+1) -> avT(p-1).
            qk_tiles = {0: emit_qk(0)} if npairs else {}
            e_store = {}
            if stage == 1 and npairs:
                q_tile, k_tile = qk_tiles[0]
                nc.sync.dma_start(out=out_d.ap()[0:128, :].bitcast(BF16)[:, :T], in_=q_tile)
                nc.sync.dma_start(out=out_d.ap()[128:256, :].bitcast(BF16)[:, :T], in_=k_tile)
            elif npairs:
                for p in range(npairs):
                    e_store[p] = emit_scores_exp(p)
                    if p + 1 < npairs:
                        qk_tiles[p + 1] = emit_qk(p + 1)
                    if p == 0:
                        emit_vt()
                    if p >= 1:
                        emit_avt(p - 1, e_store.pop(p - 1))
                emit_avt(npairs - 1, e_store.pop(npairs - 1))

        if stage == 2:
            for i in range(NCT):
                nc.sync.dma_start(out=out_d.ap()[i * 128:(i + 1) * 128, :].bitcast(BF16)[:, :T],
                                  in_=a_all[:, i, :])

        # ================= proj + residual =================
        # The residual (x + bpe, pre-folded into x_t) is accumulated into the
        # proj psum by one extra identity matmul, and the psum is DMA'd to
        # DRAM directly: no DVE work in the tail, so the tail dependency
        # chain is norm(DVE) -> transpose(DMA) -> proj(PE) -> out(DMA).
        # 256-col chunks start as soon as their two t-tile transposes land.
        mm_ctx.close()  # recycle the qk/vT mm banks for the proj pool
        with (
            tc.tile_pool(name="ps_proj", bufs=2, space="PSUM") as ps_proj,
        ):
            for ch in range(4 if stage >= 3 else 0):
                for ot in range(NCT):
                    # pad to a full PSUM bank: pool slots are packed without
                    # bank alignment, and two accumulation groups sharing a
                    # bank clobber each other (start zero-marks the bank)
                    acc = ps_proj.tile([128, 256], F32, tag="proj",
                                       padded_shape=(128, 512))
                    for kc in range(NCT):
                        nc.tensor.matmul(
                            out=acc,
                            lhsT=wpT_t[kc][:, ot * 128:(ot + 1) * 128],
                            rhs=a_all[:, kc, ch * 256:(ch + 1) * 256],
                            start=(kc == 0), stop=False)
                    nc.tensor.matmul(
                        out=acc, lhsT=ident_t,
                        rhs=x_t[ot][:, ch * 256:(ch + 1) * 256],
                        start=False, stop=True)
                    # drain psum -> sbuf on DVE early (ACT still busy with
                    # exps), on the freed-up ACT for the late chunks
                    if ch < 2:
                        nc.vector.tensor_copy(o_sb[ot][ch], acc)
                    else:
                        nc.scalar.copy(o_sb[ot][ch], acc)
                    oeng = nc.sync if ot % 2 == 0 else nc.gpsimd
                    oeng.dma_start(
                        out=out_d.ap()[ot * 128:(ot + 1) * 128, ch * 256:(ch + 1) * 256],
                        in_=o_sb[ot][ch])

    nc.finalize()
    return nc


def make_in_maps(x, gn_gamma, gn_beta, w_qkv, b_qkv, w_proj, b_proj):
    x = np.asarray(x, np.float32)
    w_qkv = np.asarray(w_qkv, np.float32)
    b_qkv = np.asarray(b_qkv, np.float32)
    w_proj = np.asarray(w_proj, np.float32)
    b_proj = np.asarray(b_proj, np.float32)

    bf = ml_dtypes.bfloat16
    wqkT = np.ascontiguousarray(w_qkv[:2 * C].T).astype(bf)      # [C, 2C]
    wvT = np.ascontiguousarray(w_qkv[2 * C:].T).astype(bf)       # [C, C]
    wpT = np.ascontiguousarray(w_proj.T).astype(bf)              # [C, C]
    bqk = np.ascontiguousarray(b_qkv[:2 * C]).reshape(2 * C, 1)
    bv = b_qkv[2 * C:]
    bpe = (b_proj + w_proj @ bv).reshape(C, 1).astype(np.float32)
    gamma = np.asarray(gn_gamma, np.float32).reshape(C, 1)
    beta = np.asarray(gn_beta, np.float32).reshape(C, 1)

    pidx = np.arange(128)
    ind8 = (pidx[:, None] // GSZ == np.arange(8)[None, :]).astype(np.float32)
    indT8 = np.ascontiguousarray(ind8.T)

    shared = {
        "wqkT": wqkT, "wvT": wvT, "wpT": wpT,
        "gamma": gamma, "beta": beta, "bqk": bqk, "bpe": np.ascontiguousarray(bpe),
        "ind8": ind8, "indT8": indT8, "ident": np.eye(128, dtype=np.float32),
    }
    xf = x.reshape(B, C, T)
    return [dict(shared, x=np.ascontiguousarray(xf[b])) for b in range(B)]


_NC_CACHE = None


def kernel(x, gn_gamma, gn_beta, w_qkv, b_qkv, w_proj, b_proj):
    global _NC_CACHE
    if _NC_CACHE is None:
        _NC_CACHE = build_nc()
    in_maps = make_in_maps(x, gn_gamma, gn_beta, w_qkv, b_qkv, w_proj, b_proj)
    res = run_bass_kernel_spmd(_NC_CACHE, in_maps, core_ids=list(range(B)))
    out = np.stack([res.results[b]["out"] for b in range(B)])
    return out.reshape(B, C, H, W).astype(np.float32)
